# revision 8
# baseline (speedup 1.0000x reference)
"""Trainium2 Bass kernel for a 2-layer LSTM (H=50) + linear head with
autoregressive future steps. Data-parallel over 8 NeuronCores (batch sharded).

Layout (per core, B_core = 2048 samples):
  - Hidden/gate channels live on SBUF partitions; batch lives on the free dim.
  - Batch is split: samples 0:1024 ("lo") use partitions 0:50, samples
    1024:2048 ("hi") use partitions 64:114 (matmul outputs at col-group 64).
  - Gate PSUM tile [128, 2048] = gates [i|f|o|g] x 512 free each; sigmoid over
    i,f,o is one ACT op spanning 3 banks.
  - Biases are folded into the matmuls via constant-1 rows appended to the
    recurrent state tiles (no separate bias adds anywhere).
  - Elementwise path in fp16 (DVE 2x mode); PSUM accumulation in fp32.

v2 changes vs v1:
  - PSUM pools assigned per HALF (pool[hf]), not per cell: cell1-h and
    cell2-h share pool[hf], so the WAR dependence (PE rewriting a gate tile
    before ACT read it) falls on the natural chain instead of stalling the
    other half's matmuls.
  - Emission is phase-split (gates both halves, then c-updates both halves,
    then h-writes both halves) so each engine's FIFO order matches data
    readiness.
  - y output in fp16 (yT fp16, host converts); removes the fp32->fp16
    convert on the feedback path and halves y DMA traffic.
"""

import sys
import os
import numpy as np

for _p in ("/opt/trn_rl_repo", "/root/.axon_site/_ro/trn_rl_repo"):
    if os.path.isdir(_p) and _p not in sys.path:
        sys.path.insert(0, _p)
        break

from contextlib import ExitStack

import concourse.bass as bass
import concourse.mybir as mybir
import concourse.tile as tile
from concourse import bacc
from concourse.bass import ds, ts
from concourse.bass_utils import run_bass_kernel_spmd

FP16 = mybir.dt.float16
FP32 = mybir.dt.float32
AF = mybir.ActivationFunctionType

H = 50
B = 16384
NCORES = 8
BC = B // NCORES          # 2048 samples per core
HALF = 1024               # samples per partition-block (lo/hi)
FREE = 512                # matmul moving free dim (one PSUM bank of fp32)

# gate order in PSUM free dim: i, f, o, g  (i,f,o share sigmoid; g is tanh)
# torch gate blocks in weights: i=0, f=1, g=2, o=3
GATE_SRC = [0, 1, 3, 2]   # our slot G -> torch block index


def _build_nc(T, F):
    # Timing-only ablation probes (wrong numerics, valid schedule shape):
    #   ABLATE=act : gate-ACT reads only 512 of 2048 free -> ACT busy drops
    #   ABLATE=chain : cell2 matmuls read the stale state tile -> chain cut
    _ab = os.environ.get("ABLATE", "")
    TT = T + F
    nc = bacc.Bacc("TRN2", target_bir_lowering=False, debug=False,
                   num_devices=NCORES)

    xT = nc.dram_tensor("xT", [T, BC], FP16, kind="ExternalInput")
    W1 = nc.dram_tensor("W1", [128, 200], FP16, kind="ExternalInput")
    W2A = nc.dram_tensor("W2A", [128, 200], FP16, kind="ExternalInput")
    W2B = nc.dram_tensor("W2B", [128, 200], FP16, kind="ExternalInput")
    WL = nc.dram_tensor("WL", [128, 1], FP16, kind="ExternalInput")
    ONES = nc.dram_tensor("ONES", [1, BC // 2], FP16, kind="ExternalInput")
    yT = nc.dram_tensor("yT", [TT, BC], FP16, kind="ExternalOutput")

    with tile.TileContext(nc) as tc, ExitStack() as ctx:
        const = ctx.enter_context(tc.tile_pool(name="const", bufs=1))
        state = ctx.enter_context(tc.tile_pool(name="state", bufs=1))
        spool = ctx.enter_context(tc.tile_pool(name="spool", bufs=4))
        tpool = ctx.enter_context(tc.tile_pool(name="tpool", bufs=4))
        ypool = ctx.enter_context(tc.tile_pool(name="ypool", bufs=2))
        pgp = [ctx.enter_context(tc.tile_pool(name=f"pg{h}", bufs=1,
                                              space="PSUM"))
               for h in range(2)]

        w1 = const.tile([128, 200], FP16, tag="w1")
        w2a = const.tile([128, 200], FP16, tag="w2a")
        w2b = const.tile([128, 200], FP16, tag="w2b")
        wl = const.tile([128, 1], FP16, tag="wl")
        nc.sync.dma_start(out=w1[:], in_=W1.ap())
        nc.sync.dma_start(out=w2a[:], in_=W2A.ap())
        nc.sync.dma_start(out=w2b[:], in_=W2B.ap())
        nc.sync.dma_start(out=wl[:], in_=WL.ap())

        # state tiles: [h1 | x | 1] and [h2 | 1] per lo/hi block, ping-pong x2
        h1x = [state.tile([128, HALF], FP16, tag=f"h1x{b}", name=f"h1x{b}")
               for b in range(2)]
        h2 = [state.tile([128, HALF], FP16, tag=f"h2{b}", name=f"h2{b}")
              for b in range(2)]
        c1 = state.tile([128, HALF], FP16, tag="c1")
        c2 = state.tile([128, HALF], FP16, tag="c2")

        for b in range(2):
            nc.vector.memset(h1x[b][:], 0.0)
            nc.vector.memset(h2[b][:], 0.0)
            # constant-1 rows (engine ops need 32-aligned partition base;
            # DMA not)
            nc.sync.dma_start(out=h1x[b][51:52, :], in_=ONES.ap())
            nc.sync.dma_start(out=h1x[b][115:116, :], in_=ONES.ap())
            nc.sync.dma_start(out=h2[b][50:51, :], in_=ONES.ap())
            nc.sync.dma_start(out=h2[b][114:115, :], in_=ONES.ap())
        nc.vector.memset(c1[:], 0.0)
        nc.vector.memset(c2[:], 0.0)

        # x for step 0
        nc.sync.dma_start(out=h1x[0][50:51, :], in_=xT.ap()[0:1, 0:HALF])
        nc.sync.dma_start(out=h1x[0][114:115, :], in_=xT.ap()[0:1, HALF:2 * HALF])

        # All gate activations are tanh: sigmoid(a) = (tanh(a/2)+1)/2 with
        # the 1/2 folded into the weights. States store 2h ("hat"), cells
        # store 2c; the STT ops below absorb the affine corrections:
        #   u1 = (f^+1)*c^ = 4 f c ;  u2 = (i^+1)*g = 2 i g
        #   c^new = 0.5*u1 + u2 = 2(fc + ig)          [STT mult/add]
        #   tch = tanh(0.5*c^new) = tanh(c_new)       [ACT scale]
        #   h^  = (o^+1)*tch = 2h                     [STT add/mult]
        MUL = mybir.AluOpType.mult
        ADD = mybir.AluOpType.add

        def gates1(hf, H1Xc):
            """cell1 matmuls + gate activations for half hf."""
            fs = ds(hf * FREE, FREE)
            pg = pgp[hf].tile([128, 2048], FP32, tag=f"pg{hf}")
            for G in range(4):
                gsl = ts(G, FREE)
                wsl = ts(G, H)
                nc.tensor.matmul(pg[0:50, gsl], w1[0:52, wsl],
                                 H1Xc[0:52, fs], start=True, stop=True)
                nc.tensor.matmul(pg[64:114, gsl], w1[64:116, wsl],
                                 H1Xc[64:116, fs], start=True, stop=True)
            s1 = spool.tile([128, 2048], FP16, tag=f"s1c1h{hf}")
            if _ab == "act":
                nc.scalar.activation(s1[0:114, 0:512], pg[0:114, 0:512],
                                     AF.Tanh)
            else:
                nc.scalar.activation(s1[0:114, :], pg[0:114, :], AF.Tanh)
            return s1

        def gates2(hf, H1Xn, H2c):
            """cell2 matmuls + gate activations for half hf."""
            fs = ds(hf * FREE, FREE)
            pg = pgp[hf].tile([128, 2048], FP32, tag=f"pg{hf}")
            for G in range(4):
                gsl = ts(G, FREE)
                wsl = ts(G, H)
                nc.tensor.matmul(pg[0:50, gsl], w2a[0:50, wsl],
                                 H1Xn[0:50, fs], start=True, stop=False)
                nc.tensor.matmul(pg[0:50, gsl], w2b[0:51, wsl],
                                 H2c[0:51, fs], start=False, stop=True)
                nc.tensor.matmul(pg[64:114, gsl], w2a[64:114, wsl],
                                 H1Xn[64:114, fs], start=True, stop=False)
                nc.tensor.matmul(pg[64:114, gsl], w2b[64:115, wsl],
                                 H2c[64:115, fs], start=False, stop=True)
            s1 = spool.tile([128, 2048], FP16, tag=f"s1c2h{hf}")
            if _ab == "act":
                nc.scalar.activation(s1[0:114, 0:512], pg[0:114, 0:512],
                                     AF.Tanh)
            else:
                nc.scalar.activation(s1[0:114, :], pg[0:114, :], AF.Tanh)
            return s1

        def cupdate(hf, cell, s1, cst):
            """c^ = 0.5*(f^+1)c^ + (i^+1)g, tch = tanh(c^/2) for half hf."""
            fs = ds(hf * FREE, FREE)
            u1 = tpool.tile([128, FREE], FP16, tag=f"u1{cell}h{hf}")
            nc.vector.scalar_tensor_tensor(
                u1[0:114, :], s1[0:114, 512:1024], 1.0, cst[0:114, fs],
                op0=ADD, op1=MUL)
            u2 = tpool.tile([128, FREE], FP16, tag=f"u2{cell}h{hf}")
            nc.vector.scalar_tensor_tensor(
                u2[0:114, :], s1[0:114, 0:512], 1.0, s1[0:114, 1536:2048],
                op0=ADD, op1=MUL)
            nc.vector.scalar_tensor_tensor(
                cst[0:114, fs], u1[0:114, :], 0.5, u2[0:114, :],
                op0=MUL, op1=ADD)
            tch = tpool.tile([128, FREE], FP16, tag=f"tc{cell}h{hf}")
            nc.scalar.activation(tch[0:114, :], cst[0:114, fs], AF.Tanh,
                                 scale=0.5)
            return tch

        def hwrite(hf, s1, tch, Hn):
            fs = ds(hf * FREE, FREE)
            nc.vector.scalar_tensor_tensor(
                Hn[0:50, fs], s1[0:50, 1024:1536], 1.0, tch[0:50, :],
                op0=ADD, op1=MUL)
            nc.vector.scalar_tensor_tensor(
                Hn[64:114, fs], s1[64:114, 1024:1536], 1.0, tch[64:114, :],
                op0=ADD, op1=MUL)

        for t in range(TT):
            cur, nxt = t % 2, (t + 1) % 2
            H1Xc, H1Xn = h1x[cur], h1x[nxt]
            H2c, H2n = h2[cur], h2[nxt]

            # cell 1
            sg1 = [gates1(hf, H1Xc) for hf in range(2)]
            tch1 = [cupdate(hf, 1, sg1[hf], c1) for hf in range(2)]
            for hf in range(2):
                hwrite(hf, sg1[hf], tch1[hf], H1Xn)

            # cell 2
            _h1src = H1Xc if _ab == "chain" else H1Xn
            sg2 = [gates2(hf, _h1src, H2c) for hf in range(2)]
            tch2 = [cupdate(hf, 2, sg2[hf], c2) for hf in range(2)]
            for hf in range(2):
                hwrite(hf, sg2[hf], tch2[hf], H2n)

            # y = Wl @ h2_t + bl; reuse pool-1 gate tile (banks 4-5)
            pgy = pgp[1].tile([128, 2048], FP32, tag="pg1")
            for hf in range(2):
                fs = ds(hf * FREE, FREE)
                nc.tensor.matmul(pgy[0:1, fs], wl[0:51, :], H2n[0:51, fs],
                                 start=True, stop=True)
                nc.tensor.matmul(pgy[32:33, fs], wl[64:115, :],
                                 H2n[64:115, fs], start=True, stop=True)
            ysb = ypool.tile([128, HALF], FP16, tag="ysb")
            nc.vector.tensor_copy(ysb[0:33, :], pgy[0:33, 0:HALF])
            nc.sync.dma_start(out=yT.ap()[t:t + 1, 0:HALF], in_=ysb[0:1, :])
            nc.sync.dma_start(out=yT.ap()[t:t + 1, HALF:2 * HALF],
                              in_=ysb[32:33, :])

            # input for step t+1
            if t + 1 < T:
                nc.sync.dma_start(out=H1Xn[50:51, :],
                                  in_=xT.ap()[t + 1:t + 2, 0:HALF])
                nc.sync.dma_start(out=H1Xn[114:115, :],
                                  in_=xT.ap()[t + 1:t + 2, HALF:2 * HALF])
            elif t + 1 < TT:
                # y feedback (already fp16)
                nc.sync.dma_start(out=H1Xn[50:51, :], in_=ysb[0:1, :])
                nc.sync.dma_start(out=H1Xn[114:115, :], in_=ysb[32:33, :])

    nc.compile()
    return nc


def _prep_weights(Wih1, Whh1, bih1, bhh1, Wih2, Whh2, bih2, bhh2, Wl, bl):
    """Weights for the tanh-unified / doubled-state kernel.

    States hold 2h, cells hold 2c. Gate pre-activations must arrive as a/2
    for i,f,o (sigmoid-via-tanh) and a for g. Each W*h term picks up another
    1/2 because the state is doubled.
    """
    b1 = (bih1 + bhh1).astype(np.float32)
    b2 = (bih2 + bhh2).astype(np.float32)

    W1 = np.zeros((128, 200), np.float32)
    W2A = np.zeros((128, 200), np.float32)
    W2B = np.zeros((128, 200), np.float32)
    WL = np.zeros((128, 1), np.float32)
    for G, src in enumerate(GATE_SRC):
        blk = slice(src * H, (src + 1) * H)
        col = slice(G * H, (G + 1) * H)
        s = 0.5 if G < 3 else 1.0   # i,f,o need tanh(a/2); g needs tanh(a)
        for base in (0, 64):
            W1[base:base + 50, col] = Whh1[blk, :].T * (0.5 * s)
            W1[base + 50, col] = Wih1[blk, 0] * s
            W1[base + 51, col] = b1[blk] * s
            W2A[base:base + 50, col] = Wih2[blk, :].T * (0.5 * s)
            W2B[base:base + 50, col] = Whh2[blk, :].T * (0.5 * s)
            W2B[base + 50, col] = b2[blk] * s
    for base in (0, 64):
        WL[base:base + 50, 0] = Wl[0, :] * 0.5
        WL[base + 50, 0] = bl[0]
    return (W1.astype(np.float16), W2A.astype(np.float16),
            W2B.astype(np.float16), WL.astype(np.float16))


_NC_CACHE = {}
_last_in_maps = None


def prepare(x, Wih1, Whh1, bih1, bhh1, Wih2, Whh2, bih2, bhh2, Wl, bl,
            future):
    """Build (cached) nc + per-core input maps + unshard fn, no execution."""
    x = np.asarray(x, np.float32)
    nB, T = x.shape
    F = int(future)
    assert nB == B, (nB, B)

    key = (T, F)
    if key not in _NC_CACHE:
        _NC_CACHE[key] = _build_nc(T, F)
    nc = _NC_CACHE[key]

    W1, W2A, W2B, WLt = _prep_weights(
        np.asarray(Wih1, np.float32), np.asarray(Whh1, np.float32),
        np.asarray(bih1, np.float32), np.asarray(bhh1, np.float32),
        np.asarray(Wih2, np.float32), np.asarray(Whh2, np.float32),
        np.asarray(bih2, np.float32), np.asarray(bhh2, np.float32),
        np.asarray(Wl, np.float32), np.asarray(bl, np.float32))

    in_maps = []
    for c in range(NCORES):
        xc = np.ascontiguousarray(x[c * BC:(c + 1) * BC, :].T).astype(np.float16)
        in_maps.append({"xT": xc, "W1": W1, "W2A": W2A, "W2B": W2B,
                        "WL": WLt, "ONES": np.ones((1, BC // 2), np.float16)})

    def unshard(results):
        out = np.empty((B, T + F), np.float32)
        for c in range(NCORES):
            out[c * BC:(c + 1) * BC, :] = results[c]["yT"].T.astype(np.float32)
        return out

    return nc, in_maps, unshard


def _run(trace=False, **kin):
    nc, in_maps, unshard = prepare(**kin)
    global _last_in_maps
    _last_in_maps = in_maps
    res = run_bass_kernel_spmd(nc, in_maps, list(range(NCORES)), trace=trace)
    return unshard(res.results), res


def kernel(**inputs):
    out, _ = _run(**inputs)
    return out
